# revision 6
# baseline (speedup 1.0000x reference)
"""Trainium2 Bass kernel for DifferentiableGMM log-likelihood (v3).

Computes  out[n] = logsumexp_k( -0.5*||(x[n]-mu[k])/s[k]||^2 - log|s[k]| + log w[k] )
for N=2,000,000 points, K=16 diagonal-covariance components, D=3.

v3: v2's software-pipelined fp16 feature path + trimmed geometry.
Per core 1984 j-columns (tiles 512,512,512,448; the last pair of tile 3
is 64-j wide) = 253,952 points, 1.55% padding waste vs 4.64% at 2048.

Per pair q (width w in {128, 64} j-columns x 128 partitions):
  DVE builds F = [x^2, x, x0x1-pad] fp16; PE transposes w/16 blocks of
  [128,128] into tp fp16 PSUM; DVE copies tp -> ft SBUF (fp16 2x rate);
  per batch (2 per pair) 2 W-matmuls (fp16 Wdiag block-diag over 8
  j-offsets x 16 components) -> m2 f32 PSUM; ACT exp(+c_k bias) -> e2;
  2 ones-matmuls (lagged GMM_LAG batches so the PE never waits on ACT)
  accumulate 16 rounds into sums [128,512] PSUM; ACT Ln -> DMA out.

The GMM_REPS timing loop is unrolled GMM_UNROLL bodies per For_i
iteration to amortize the all-engine loop barrier; ring-buffered tile
pools pipeline body-to-body.
"""

import os
import numpy as np

K = 16
D = 3
EPS = 1e-6
N_CORES = 8
N_FULL = 2_000_000

TILE_W = tuple(int(w) for w in os.environ.get(
    "GMM_TILEW", "512,512,512,448").split(","))  # j-columns per tile
T_TILES = len(TILE_W)
W_TOTAL = sum(TILE_W)                # 1984
NPC = 128 * W_TOTAL                  # 253952 points per core
N_PAD = N_CORES * NPC                # 2031616

# pairs: (tile, j_off within tile, width)
PAIRS = []
for _t, _w in enumerate(TILE_W):
    _off = 0
    while _off < _w:
        _pw = min(128, _w - _off)
        PAIRS.append((_t, _off, _pw))
        _off += _pw
N_PAIRS = len(PAIRS)                 # 16

_compiled_cache = {}


def _build_nc():
    reps = int(os.environ.get("GMM_REPS", "1"))
    lag = int(os.environ.get("GMM_LAG", "5"))
    warm = int(os.environ.get("GMM_WARM", "10"))
    unroll = int(os.environ.get("GMM_UNROLL", "5"))

    # Force the ACT-table chooser to the set that holds Exp, Ln AND Copy
    # together, so no table reloads happen mid-kernel.
    import concourse.bacc as _bacc_mod
    from concourse.hw_specs import get_activation_tables as _orig_gat
    def _only_combined(arch, __orig=_orig_gat):
        return {name: (fns if name == "natural_log_exp_and_others" else set())
                for name, fns in __orig(arch).items()}
    _bacc_mod.get_activation_tables = _only_combined

    import concourse.bacc as bacc
    import concourse.mybir as mybir
    import concourse.tile as tile
    from concourse._compat import get_trn_type

    f32 = mybir.dt.float32
    f32r = mybir.dt.float32r
    f16 = mybir.dt.float16
    bf16 = mybir.dt.bfloat16
    AF = mybir.ActivationFunctionType
    e2dt = bf16 if int(os.environ.get("GMM_E2BF16", "1")) else f32r

    nc = bacc.Bacc(
        get_trn_type() or "TRN2",
        target_bir_lowering=False,
        debug=False,
        num_devices=N_CORES,
    )

    x_drams = [nc.dram_tensor(f"x{t}", [128, 3 * TILE_W[t]], f32,
                              kind="ExternalInput") for t in range(T_TILES)]
    wdiag_dram = nc.dram_tensor("wdiag", [128, 128], f16, kind="ExternalInput")
    cvec_dram = nc.dram_tensor("cvec", [128, 1], f32, kind="ExternalInput")
    ones_dram = nc.dram_tensor("onesbig", [128, 256],
                               bf16 if int(os.environ.get("GMM_E2BF16", "1"))
                               else f32r, kind="ExternalInput")
    ident_dram = nc.dram_tensor("ident", [128, 128], f16, kind="ExternalInput")
    out_dram = nc.dram_tensor("out", [T_TILES, 128, 512], f32,
                              kind="ExternalOutput")

    with tile.TileContext(nc) as tc:
        with (
            tc.tile_pool(name="singles", bufs=1) as singles,
            tc.tile_pool(name="xin", bufs=6) as xin_pool,
            tc.tile_pool(name="fpool", bufs=4) as f_pool,
            tc.tile_pool(name="ftp", bufs=3) as ft_pool,
            tc.tile_pool(name="etile", bufs=2 + lag) as e_pool,
            tc.tile_pool(name="osb", bufs=2) as out_pool,
            tc.tile_pool(name="tpsum", bufs=2, space="PSUM") as tpsum_pool,
            tc.tile_pool(name="mpsum", bufs=2, space="PSUM") as mpsum_pool,
            tc.tile_pool(name="spsum", bufs=2, space="PSUM") as spsum_pool,
        ):
            x_tiles = [None] * N_PAIRS

            def dma_x(q, name):
                t, joff, w = PAIRS[q]
                x_sb = xin_pool.tile([128, 384], f32, tag="x", name=name)
                nc.sync.dma_start(x_sb[:, 0:3 * w],
                                  x_drams[t].ap()[:, 3 * joff:3 * (joff + w)])
                x_tiles[q] = x_sb

            # Constants staged through compute-engine copies so consumers'
            # waits merge into their existing sem domains (matmuls allow
            # only ONE sync wait).  ident first: it gates the PE stream.
            Wdiag_st = singles.tile([128, 128], f16)
            cvec_st = singles.tile([128, 1], f32)
            ones_st = singles.tile([128, 256],
                                   bf16 if int(os.environ.get("GMM_E2BF16", "1"))
                                   else f32r)
            ident_st = singles.tile([128, 128], f16)
            nc.sync.dma_start(ident_st[:], ident_dram[:, :])
            nc.sync.dma_start(Wdiag_st[:], wdiag_dram[:, :])
            nc.sync.dma_start(cvec_st[:], cvec_dram[:, :])
            nc.sync.dma_start(ones_st[:], ones_dram[:, :])
            # first pairs of x behind the constants
            for _q in range(4):
                dma_x(_q, f"x_{_q}")
            Wdiag = singles.tile([128, 128], f16)
            cvec = singles.tile([128, 1], f32)
            ones_big = singles.tile([128, 256],
                                    bf16 if int(os.environ.get("GMM_E2BF16", "1"))
                                    else f32r)
            identF = singles.tile([128, 128], f16)
            nc.vector.tensor_copy(identF[:], ident_st[:])
            nc.vector.tensor_copy(Wdiag[:], Wdiag_st[:])
            nc.scalar.copy(ones_big[:], ones_st[:])
            nc.scalar.copy(cvec[:], cvec_st[:])

            # ---- PE warmup (once) under the initial DMA ----
            for _ in range(warm):
                tp_w = tpsum_pool.tile([128, 128], f16, tag="tp")
                nc.tensor.transpose(tp_w[:], identF[:], identF[:])

            def main_body():
                f_tiles = [None] * N_PAIRS
                ft_tiles = [None] * N_PAIRS
                m2_tiles = [None] * (2 * N_PAIRS)
                e2_tiles = [None] * (2 * N_PAIRS)
                sums_tiles = [None] * T_TILES
                ones_issued = 0

                def fbuild(q):
                    w = PAIRS[q][2]
                    xg2 = x_tiles[q][:, 0:3 * w].rearrange(
                        "p (j d) -> p j d", d=D)
                    F = f_pool.tile([128, 128, 8], f16, tag="F", name=f"F_{q}")
                    nc.vector.tensor_mul(F[:, 0:w, 0:3], xg2, xg2)
                    nc.vector.tensor_copy(F[:, 0:w, 3:6], xg2)
                    # pad cols 6,7 with x0,x1 (their Wdiag rows are 0)
                    nc.vector.tensor_copy(F[:, 0:w, 6:8], xg2[:, :, 0:2])
                    f_tiles[q] = F

                def transposes(q):
                    w = PAIRS[q][2]
                    Fflat = f_tiles[q][:].rearrange("p j c -> p (j c)")
                    tp = tpsum_pool.tile([128, 1024], f16, tag="tp",
                                         name=f"tp_{q}")
                    for u in range(w // 16):
                        nc.tensor.transpose(
                            tp[:, 128 * u:128 * u + 128],
                            Fflat[:, 128 * u:128 * u + 128],
                            identF[:],
                        )
                    return tp

                def ftcopy(q, tp):
                    w = PAIRS[q][2]
                    ft = ft_pool.tile([128, 1024], f16, tag="ft", name=f"ft_{q}")
                    nc.vector.tensor_copy(ft[:, 0:8 * w], tp[:, 0:8 * w])
                    ft_tiles[q] = ft

                def wmm(g):
                    q, bi = g // 2, g % 2
                    w = PAIRS[q][2]
                    fw = 4 * w          # free width per half-batch
                    ft = ft_tiles[q]
                    m2 = mpsum_pool.tile([128, 1024], f32, tag="m2",
                                         name=f"m2_{g}")
                    for h in range(2):
                        rows = slice(64 * h, 64 * h + 64)
                        # PSUM matmul outputs must be bank-aligned: place
                        # half h at element offset 512*h (bank start), not
                        # fw*h
                        nc.tensor.matmul(
                            m2[:, 512 * h:512 * h + fw],
                            Wdiag[rows, :],
                            ft[rows, fw * bi:fw * bi + fw],
                            start=True, stop=True)
                    m2_tiles[g] = m2

                def expb(g):
                    q = g // 2
                    fw = 4 * PAIRS[q][2]
                    e2 = e_pool.tile([128, 1024], e2dt, tag="e2", name=f"e2_{g}")
                    if fw == 512:
                        nc.scalar.activation(e2[:], m2_tiles[g][:],
                                             AF.Exp, bias=cvec[:], scale=1.0)
                    else:
                        for h in range(2):
                            nc.scalar.activation(
                                e2[:, 512 * h:512 * h + fw],
                                m2_tiles[g][:, 512 * h:512 * h + fw],
                                AF.Exp, bias=cvec[:], scale=1.0)
                    e2_tiles[g] = e2

                def onesb(g):
                    q, bi = g // 2, g % 2
                    t, _, w = PAIRS[q]
                    fw = 4 * w
                    qi = sum(1 for (tt, _, _) in PAIRS[:q] if tt == t)
                    if sums_tiles[t] is None:
                        sums_tiles[t] = spsum_pool.tile(
                            [128, 512], f32, tag="sums", name=f"sums_{t}")
                    e2 = e2_tiles[g]
                    for h in range(2):
                        s = 4 * qi + 2 * bi + h
                        nc.tensor.matmul(
                            sums_tiles[t][:, 0:fw],
                            ones_big[:, 120 - 8 * s:248 - 8 * s],
                            e2[:, 512 * h:512 * h + fw],
                            start=(s == 0), stop=(s == 15))

                def lnout(t):
                    out_sb = out_pool.tile([128, 512], f32, tag="o", name=f"o_{t}")
                    if TILE_W[t] < 512:
                        # sums[96:,256:] is never written by the s-chain for
                        # a 448-wide tile; Ln and DMA only the valid regions
                        # (pre-zeroed DRAM output covers the rest)
                        nc.scalar.activation(out_sb[:, 0:256],
                                             sums_tiles[t][:, 0:256], AF.Ln)
                        nc.scalar.activation(out_sb[0:96, 256:512],
                                             sums_tiles[t][0:96, 256:512], AF.Ln)
                        nc.sync.dma_start(out_dram.ap()[t][:, 0:256],
                                          out_sb[:, 0:256])
                        nc.sync.dma_start(out_dram.ap()[t][0:96, 256:512],
                                          out_sb[0:96, 256:512])
                    else:
                        nc.scalar.activation(out_sb[:], sums_tiles[t][:], AF.Ln)
                        nc.sync.dma_start(out_dram.ap()[t], out_sb[:])

                last_g_of_tile = {}
                for g in range(2 * N_PAIRS):
                    last_g_of_tile[PAIRS[g // 2][0]] = g

                # ---- pipelined emission ----
                fbuild(0)
                fbuild(1)
                for q in range(N_PAIRS):
                    if q + 4 < N_PAIRS:
                        dma_x(q + 4, f"x_{q + 4}")
                    if q + 2 < N_PAIRS:
                        fbuild(q + 2)
                    tp = transposes(q)
                    ftcopy(q, tp)
                    for bi in range(2):
                        g = 2 * q + bi
                        wmm(g)
                        expb(g)
                        go = g - lag
                        if go >= 0:
                            onesb(go)
                            ones_issued = go + 1
                            t = PAIRS[go // 2][0]
                            if go == last_g_of_tile[t]:
                                lnout(t)
                # refresh x(0..3) for the next rep (same emission-indexed slots)
                for _q in range(4):
                    dma_x(_q, f"x_{_q}r")
                for go in range(ones_issued, 2 * N_PAIRS):
                    onesb(go)
                    t = PAIRS[go // 2][0]
                    if go == last_g_of_tile[t]:
                        lnout(t)

            if reps == 1:
                main_body()
            else:
                n_extra = reps - 1
                assert n_extra % unroll == 0, (reps, unroll)
                main_body()
                with tc.For_i(0, n_extra // unroll, 1):
                    for _ in range(unroll):
                        main_body()

    nc.compile()
    return nc


def _scatter_indices():
    """For each (t, q, f) output position: the core-local point index, or -1."""
    W_pre = np.cumsum([0] + list(TILE_W))
    idx = np.full((T_TILES, 128, 512), -1, dtype=np.int64)
    for t, wt in enumerate(TILE_W):
        pairs_t = [(joff, w) for (tt, joff, w) in PAIRS if tt == t]
        qv, fv = np.meshgrid(np.arange(128), np.arange(512), indexing="ij")
        B = qv // 16
        h = (qv // 8) % 2
        bp = qv % 8
        up = fv // 128
        px = fv % 128
        qi = B // 2
        bi = B % 2
        for (pi, (joff, w)) in enumerate(pairs_t):
            sel = (qi == pi) & (up < w // 32)
            j = joff + (w // 2) * bi + 16 * up + 8 * h + bp
            n = 128 * W_pre[t] + px * wt + j
            idx[t][sel] = n[sel]
    return idx.reshape(-1)


def _host_constants(means, covariances, weights):
    """Wdiag [128,128] f16, cvec [128,1], ones_big [128,256], ident f16."""
    covp = covariances.astype(np.float64) + EPS
    mu = means.astype(np.float64)
    A = -0.5 / covp                              # [K,D] coeff of x^2
    B = mu / covp                                # [K,D] coeff of x
    c_k = (-0.5 * (mu * mu / covp).sum(1) - 0.5 * np.log(covp).sum(1)
           - 0.5 * D * np.log(2 * np.pi) + np.log(weights.astype(np.float64)))

    coefT = np.zeros((8, K), np.float32)
    coefT[0:3] = A.T
    coefT[3:6] = B.T
    wd8 = np.zeros((64, 128), np.float32)
    for b in range(8):
        wd8[8 * b:8 * b + 8, 16 * b:16 * b + 16] = coefT
    wdiag = np.concatenate([wd8, wd8], 0).astype(np.float16)

    cvec = np.tile(c_k.astype(np.float32), 8).reshape(128, 1)

    ones_big = np.zeros((128, 256), np.float32)
    for b in range(8):
        ones_big[16 * b:16 * b + 16, 120 + b] = 1.0
    if int(os.environ.get("GMM_E2BF16", "1")):
        import ml_dtypes
        ones_big = ones_big.astype(ml_dtypes.bfloat16)

    ident = np.eye(128, dtype=np.float16)
    return wdiag, cvec, ones_big, ident


def _in_maps(x_pad, wdiag, cvec, ones_big, ident):
    W_pre = np.cumsum([0] + list(TILE_W))
    maps = []
    for c in range(N_CORES):
        shard = x_pad[c * NPC:(c + 1) * NPC]         # [NPC, 3]
        m = {"wdiag": wdiag, "cvec": cvec, "onesbig": ones_big, "ident": ident}
        for t, wt in enumerate(TILE_W):
            xt = shard[128 * W_pre[t]:128 * W_pre[t + 1]]
            m[f"x{t}"] = np.ascontiguousarray(
                xt.reshape(128, 3 * wt))
        maps.append(m)
    return maps


def kernel(x, means, covariances, weights):
    from concourse.bass_utils import run_bass_kernel_spmd

    x = np.ascontiguousarray(np.asarray(x, dtype=np.float32))
    means = np.ascontiguousarray(np.asarray(means, dtype=np.float32))
    covariances = np.ascontiguousarray(np.asarray(covariances, dtype=np.float32))
    weights = np.ascontiguousarray(np.asarray(weights, dtype=np.float32)).reshape(K)

    n = x.shape[0]
    x_pad = np.zeros((N_PAD, D), dtype=np.float32)
    x_pad[:n] = x

    key = "nc"
    if key not in _compiled_cache:
        _compiled_cache[key] = _build_nc()
    nc = _compiled_cache[key]

    wdiag, cvec, ones_big, ident = _host_constants(means, covariances, weights)
    in_maps = _in_maps(x_pad, wdiag, cvec, ones_big, ident)

    res = run_bass_kernel_spmd(
        nc, in_maps, core_ids=list(range(N_CORES)),
        trace=bool(int(os.environ.get("GMM_TRACE", "0"))),
    )
    kernel.last_results = res

    idx = _scatter_indices()
    valid = idx >= 0
    out_pad = np.empty(N_PAD, dtype=np.float32)
    for c in range(N_CORES):
        raw = res.results[c]["out"].reshape(-1)
        out_pad[c * NPC + idx[valid]] = raw[valid]
    return out_pad[:n]


# revision 8
# speedup vs baseline: 1.0033x; 1.0033x over previous
"""Trainium2 Bass kernel for DifferentiableGMM log-likelihood (v3).

Computes  out[n] = logsumexp_k( -0.5*||(x[n]-mu[k])/s[k]||^2 - log|s[k]| + log w[k] )
for N=2,000,000 points, K=16 diagonal-covariance components, D=3.

v3: v2's software-pipelined fp16 feature path + trimmed geometry.
Per core 1984 j-columns (tiles 512,512,512,448; the last pair of tile 3
is 64-j wide) = 253,952 points, 1.55% padding waste vs 4.64% at 2048.

Per pair q (width w in {128, 64} j-columns x 128 partitions):
  DVE builds F = [x^2, x, x0x1-pad] fp16; PE transposes w/16 blocks of
  [128,128] into tp fp16 PSUM; DVE copies tp -> ft SBUF (fp16 2x rate);
  per batch (2 per pair) 2 W-matmuls (fp16 Wdiag block-diag over 8
  j-offsets x 16 components) -> m2 f32 PSUM; ACT exp(+c_k bias) -> e2;
  2 ones-matmuls (lagged GMM_LAG batches so the PE never waits on ACT)
  accumulate 16 rounds into sums [128,512] PSUM; ACT Ln -> DMA out.

The GMM_REPS timing loop is unrolled GMM_UNROLL bodies per For_i
iteration to amortize the all-engine loop barrier; ring-buffered tile
pools pipeline body-to-body.
"""

import os
import numpy as np

K = 16
D = 3
EPS = 1e-6
N_CORES = 8
N_FULL = 2_000_000

TILE_W = tuple(int(w) for w in os.environ.get(
    "GMM_TILEW", "512,512,512,448").split(","))  # j-columns per tile
T_TILES = len(TILE_W)
W_TOTAL = sum(TILE_W)                # 1984
NPC = 128 * W_TOTAL                  # 253952 points per core
N_PAD = N_CORES * NPC                # 2031616

# pairs: (tile, j_off within tile, width)
PAIRS = []
for _t, _w in enumerate(TILE_W):
    _off = 0
    while _off < _w:
        _pw = min(128, _w - _off)
        PAIRS.append((_t, _off, _pw))
        _off += _pw
N_PAIRS = len(PAIRS)                 # 16

_compiled_cache = {}


def _build_nc():
    reps = int(os.environ.get("GMM_REPS", "1"))
    lag = int(os.environ.get("GMM_LAG", "5"))
    warm = int(os.environ.get("GMM_WARM", "10"))
    unroll = int(os.environ.get("GMM_UNROLL", "5"))

    # Force the ACT-table chooser to the set that holds Exp, Ln AND Copy
    # together, so no table reloads happen mid-kernel.
    import concourse.bacc as _bacc_mod
    from concourse.hw_specs import get_activation_tables as _orig_gat
    def _only_combined(arch, __orig=_orig_gat):
        return {name: (fns if name == "natural_log_exp_and_others" else set())
                for name, fns in __orig(arch).items()}
    _bacc_mod.get_activation_tables = _only_combined

    import concourse.bacc as bacc
    import concourse.mybir as mybir
    import concourse.tile as tile
    from concourse._compat import get_trn_type

    f32 = mybir.dt.float32
    f32r = mybir.dt.float32r
    f16 = mybir.dt.float16
    bf16 = mybir.dt.bfloat16
    AF = mybir.ActivationFunctionType
    e2dt = bf16 if int(os.environ.get("GMM_E2BF16", "1")) else f32r

    nc = bacc.Bacc(
        get_trn_type() or "TRN2",
        target_bir_lowering=False,
        debug=False,
        num_devices=N_CORES,
    )

    x_drams = [nc.dram_tensor(f"x{t}", [128, 3 * TILE_W[t]], f32,
                              kind="ExternalInput") for t in range(T_TILES)]
    wdiag_dram = nc.dram_tensor("wdiag", [128, 128], f16, kind="ExternalInput")
    cvec_dram = nc.dram_tensor("cvec", [128, 1], f32, kind="ExternalInput")
    ones_dram = nc.dram_tensor("onesbig", [128, 256],
                               bf16 if int(os.environ.get("GMM_E2BF16", "1"))
                               else f32r, kind="ExternalInput")
    ident_dram = nc.dram_tensor("ident", [128, 128], f16, kind="ExternalInput")
    out_dram = nc.dram_tensor("out", [T_TILES, 128, 512], f32,
                              kind="ExternalOutput")

    with tile.TileContext(nc) as tc:
        with (
            tc.tile_pool(name="singles", bufs=1) as singles,
            tc.tile_pool(name="xin", bufs=6) as xin_pool,
            tc.tile_pool(name="fpool", bufs=4) as f_pool,
            tc.tile_pool(name="ftp", bufs=3) as ft_pool,
            tc.tile_pool(name="etile", bufs=2 + lag) as e_pool,
            tc.tile_pool(name="osb", bufs=2) as out_pool,
            tc.tile_pool(name="tpsum", bufs=2, space="PSUM") as tpsum_pool,
            tc.tile_pool(name="mpsum", bufs=2, space="PSUM") as mpsum_pool,
            tc.tile_pool(name="spsum", bufs=2, space="PSUM") as spsum_pool,
        ):
            x_tiles = [None] * N_PAIRS

            def dma_x(q, name):
                t, joff, w = PAIRS[q]
                x_sb = xin_pool.tile([128, 384], f32, tag="x", name=name)
                nc.sync.dma_start(x_sb[:, 0:3 * w],
                                  x_drams[t].ap()[:, 3 * joff:3 * (joff + w)])
                x_tiles[q] = x_sb

            # Constants staged through compute-engine copies so consumers'
            # waits merge into their existing sem domains (matmuls allow
            # only ONE sync wait).  ident first: it gates the PE stream.
            Wdiag_st = singles.tile([128, 128], f16)
            cvec_st = singles.tile([128, 1], f32)
            ones_st = singles.tile([128, 256],
                                   bf16 if int(os.environ.get("GMM_E2BF16", "1"))
                                   else f32r)
            ident_st = singles.tile([128, 128], f16)
            nc.sync.dma_start(ident_st[:], ident_dram[:, :])
            nc.sync.dma_start(Wdiag_st[:], wdiag_dram[:, :])
            nc.sync.dma_start(cvec_st[:], cvec_dram[:, :])
            nc.sync.dma_start(ones_st[:], ones_dram[:, :])
            # first pairs of x behind the constants
            for _q in range(4):
                dma_x(_q, f"x_{_q}")
            Wdiag = singles.tile([128, 128], f16)
            cvec = singles.tile([128, 1], f32)
            ones_big = singles.tile([128, 256],
                                    bf16 if int(os.environ.get("GMM_E2BF16", "1"))
                                    else f32r)
            identF = singles.tile([128, 128], f16)
            nc.vector.tensor_copy(identF[:], ident_st[:])
            nc.vector.tensor_copy(Wdiag[:], Wdiag_st[:])
            nc.scalar.copy(ones_big[:], ones_st[:])
            nc.scalar.copy(cvec[:], cvec_st[:])

            # ---- PE warmup (once) under the initial DMA ----
            for _ in range(warm):
                tp_w = tpsum_pool.tile([128, 128], f16, tag="tp")
                nc.tensor.transpose(tp_w[:], identF[:], identF[:])

            def main_body():
                f_tiles = [None] * N_PAIRS
                ft_tiles = [None] * N_PAIRS
                m2_tiles = [None] * (2 * N_PAIRS)
                e2_tiles = [None] * (2 * N_PAIRS)
                sums_tiles = [None] * T_TILES
                ones_issued = 0

                def fbuild(q):
                    w = PAIRS[q][2]
                    xg2 = x_tiles[q][:, 0:3 * w].rearrange(
                        "p (j d) -> p j d", d=D)
                    F = f_pool.tile([128, 128, 8], f16, tag="F", name=f"F_{q}")
                    nc.vector.tensor_mul(F[:, 0:w, 0:3], xg2, xg2)
                    nc.vector.tensor_copy(F[:, 0:w, 3:6], xg2)
                    # pad cols 6,7 with x0,x1 (their Wdiag rows are 0)
                    nc.vector.tensor_copy(F[:, 0:w, 6:8], xg2[:, :, 0:2])
                    f_tiles[q] = F

                def transposes(q):
                    w = PAIRS[q][2]
                    Fflat = f_tiles[q][:].rearrange("p j c -> p (j c)")
                    tp = tpsum_pool.tile([128, 1024], f16, tag="tp",
                                         name=f"tp_{q}")
                    for u in range(w // 16):
                        nc.tensor.transpose(
                            tp[:, 128 * u:128 * u + 128],
                            Fflat[:, 128 * u:128 * u + 128],
                            identF[:],
                        )
                    return tp

                def ftcopy(q, tp):
                    w = PAIRS[q][2]
                    ft = ft_pool.tile([128, 1024], f16, tag="ft", name=f"ft_{q}")
                    nc.vector.tensor_copy(ft[:, 0:8 * w], tp[:, 0:8 * w])
                    ft_tiles[q] = ft

                def wmm(g):
                    q, bi = g // 2, g % 2
                    w = PAIRS[q][2]
                    fw = 4 * w          # free width per half-batch
                    ft = ft_tiles[q]
                    m2 = mpsum_pool.tile([128, 1024], f32, tag="m2",
                                         name=f"m2_{g}")
                    for h in range(2):
                        rows = slice(64 * h, 64 * h + 64)
                        # PSUM matmul outputs must be bank-aligned: place
                        # half h at element offset 512*h (bank start), not
                        # fw*h
                        nc.tensor.matmul(
                            m2[:, 512 * h:512 * h + fw],
                            Wdiag[rows, :],
                            ft[rows, fw * bi:fw * bi + fw],
                            start=True, stop=True)
                    m2_tiles[g] = m2

                def expb(g):
                    q = g // 2
                    fw = 4 * PAIRS[q][2]
                    e2 = e_pool.tile([128, 1024], e2dt, tag="e2", name=f"e2_{g}")
                    if fw == 512:
                        nc.scalar.activation(e2[:], m2_tiles[g][:],
                                             AF.Exp, bias=cvec[:], scale=1.0)
                    else:
                        for h in range(2):
                            nc.scalar.activation(
                                e2[:, 512 * h:512 * h + fw],
                                m2_tiles[g][:, 512 * h:512 * h + fw],
                                AF.Exp, bias=cvec[:], scale=1.0)
                    e2_tiles[g] = e2

                def onesb(g):
                    q, bi = g // 2, g % 2
                    t, _, w = PAIRS[q]
                    fw = 4 * w
                    qi = sum(1 for (tt, _, _) in PAIRS[:q] if tt == t)
                    if sums_tiles[t] is None:
                        sums_tiles[t] = spsum_pool.tile(
                            [128, 512], f32, tag="sums", name=f"sums_{t}")
                    e2 = e2_tiles[g]
                    for h in range(2):
                        s = 4 * qi + 2 * bi + h
                        nc.tensor.matmul(
                            sums_tiles[t][:, 0:fw],
                            ones_big[:, 120 - 8 * s:248 - 8 * s],
                            e2[:, 512 * h:512 * h + fw],
                            start=(s == 0), stop=(s == 15))

                def lnout(t):
                    out_sb = out_pool.tile([128, 512], f32, tag="o", name=f"o_{t}")
                    if TILE_W[t] < 512:
                        # sums[96:,256:] is never written by the s-chain for
                        # a 448-wide tile; Ln and DMA only the valid regions
                        # (pre-zeroed DRAM output covers the rest)
                        nc.scalar.activation(out_sb[:, 0:256],
                                             sums_tiles[t][:, 0:256], AF.Ln)
                        nc.scalar.activation(out_sb[0:96, 256:512],
                                             sums_tiles[t][0:96, 256:512], AF.Ln)
                        nc.sync.dma_start(out_dram.ap()[t][:, 0:256],
                                          out_sb[:, 0:256])
                        nc.sync.dma_start(out_dram.ap()[t][0:96, 256:512],
                                          out_sb[0:96, 256:512])
                    else:
                        nc.scalar.activation(out_sb[:], sums_tiles[t][:], AF.Ln)
                        nc.sync.dma_start(out_dram.ap()[t], out_sb[:])

                last_g_of_tile = {}
                for g in range(2 * N_PAIRS):
                    last_g_of_tile[PAIRS[g // 2][0]] = g

                # ---- pipelined emission ----
                fbuild(0)
                fbuild(1)
                for q in range(N_PAIRS):
                    if q + 4 < N_PAIRS:
                        dma_x(q + 4, f"x_{q + 4}")
                    if q + 2 < N_PAIRS:
                        fbuild(q + 2)
                    tp = transposes(q)
                    ftcopy(q, tp)
                    for bi in range(2):
                        g = 2 * q + bi
                        wmm(g)
                        expb(g)
                        go = g - lag
                        if go >= 0:
                            onesb(go)
                            ones_issued = go + 1
                            t = PAIRS[go // 2][0]
                            if go == last_g_of_tile[t]:
                                lnout(t)
                # refresh x(0..3) for the next rep (same emission-indexed slots)
                for _q in range(4):
                    dma_x(_q, f"x_{_q}r")
                for go in range(ones_issued, 2 * N_PAIRS):
                    onesb(go)
                    t = PAIRS[go // 2][0]
                    if go == last_g_of_tile[t]:
                        lnout(t)

            if reps == 1:
                main_body()
            else:
                n_extra = reps - 1
                assert n_extra % unroll == 0, (reps, unroll)
                main_body()
                with tc.For_i(0, n_extra // unroll, 1):
                    for _ in range(unroll):
                        main_body()

    nc.compile()
    return nc


def _scatter_indices():
    """For each (t, q, f) output position: the core-local point index, or -1."""
    W_pre = np.cumsum([0] + list(TILE_W))
    idx = np.full((T_TILES, 128, 512), -1, dtype=np.int64)
    for t, wt in enumerate(TILE_W):
        pairs_t = [(joff, w) for (tt, joff, w) in PAIRS if tt == t]
        qv, fv = np.meshgrid(np.arange(128), np.arange(512), indexing="ij")
        B = qv // 16
        h = (qv // 8) % 2
        bp = qv % 8
        up = fv // 128
        px = fv % 128
        qi = B // 2
        bi = B % 2
        for (pi, (joff, w)) in enumerate(pairs_t):
            sel = (qi == pi) & (up < w // 32)
            j = joff + (w // 2) * bi + 16 * up + 8 * h + bp
            n = 128 * W_pre[t] + px * wt + j
            idx[t][sel] = n[sel]
    return idx.reshape(-1)


def _host_constants(means, covariances, weights):
    """Wdiag [128,128] f16, cvec [128,1], ones_big [128,256], ident f16."""
    covp = covariances.astype(np.float64) + EPS
    mu = means.astype(np.float64)
    A = -0.5 / covp                              # [K,D] coeff of x^2
    B = mu / covp                                # [K,D] coeff of x
    c_k = (-0.5 * (mu * mu / covp).sum(1) - 0.5 * np.log(covp).sum(1)
           - 0.5 * D * np.log(2 * np.pi) + np.log(weights.astype(np.float64)))

    coefT = np.zeros((8, K), np.float32)
    coefT[0:3] = A.T
    coefT[3:6] = B.T
    wd8 = np.zeros((64, 128), np.float32)
    for b in range(8):
        wd8[8 * b:8 * b + 8, 16 * b:16 * b + 16] = coefT
    wdiag = np.concatenate([wd8, wd8], 0).astype(np.float16)

    cvec = np.tile(c_k.astype(np.float32), 8).reshape(128, 1)

    ones_big = np.zeros((128, 256), np.float32)
    for b in range(8):
        ones_big[16 * b:16 * b + 16, 120 + b] = 1.0
    if int(os.environ.get("GMM_E2BF16", "1")):
        import ml_dtypes
        ones_big = ones_big.astype(ml_dtypes.bfloat16)

    ident = np.eye(128, dtype=np.float16)
    return wdiag, cvec, ones_big, ident


def _in_maps(x_pad, wdiag, cvec, ones_big, ident):
    W_pre = np.cumsum([0] + list(TILE_W))
    maps = []
    for c in range(N_CORES):
        shard = x_pad[c * NPC:(c + 1) * NPC]         # [NPC, 3]
        m = {"wdiag": wdiag, "cvec": cvec, "onesbig": ones_big, "ident": ident}
        for t, wt in enumerate(TILE_W):
            xt = shard[128 * W_pre[t]:128 * W_pre[t + 1]]
            m[f"x{t}"] = np.ascontiguousarray(
                xt.reshape(128, 3 * wt))
        maps.append(m)
    return maps


def kernel(x, means, covariances, weights):
    from concourse.bass_utils import run_bass_kernel_spmd

    x = np.ascontiguousarray(np.asarray(x, dtype=np.float32))
    means = np.ascontiguousarray(np.asarray(means, dtype=np.float32))
    covariances = np.ascontiguousarray(np.asarray(covariances, dtype=np.float32))
    weights = np.ascontiguousarray(np.asarray(weights, dtype=np.float32)).reshape(K)

    n = x.shape[0]
    x_pad = np.zeros((N_PAD, D), dtype=np.float32)
    x_pad[:n] = x

    key = "nc"
    if key not in _compiled_cache:
        _compiled_cache[key] = _build_nc()
    nc = _compiled_cache[key]

    wdiag, cvec, ones_big, ident = _host_constants(means, covariances, weights)
    in_maps = _in_maps(x_pad, wdiag, cvec, ones_big, ident)

    res = run_bass_kernel_spmd(
        nc, in_maps, core_ids=list(range(N_CORES)),
        trace=bool(int(os.environ.get("GMM_TRACE", "0"))),
    )
    kernel.last_results = res

    idx = _scatter_indices()
    valid = idx >= 0
    out_pad = np.empty(N_PAD, dtype=np.float32)
    for c in range(N_CORES):
        raw = res.results[c]["out"].reshape(-1)
        out_pad[c * NPC + idx[valid]] = raw[valid]
    return out_pad[:n]
